# revision 2
# baseline (speedup 1.0000x reference)
"""Causal single-head attention on 8 TRN2 NeuronCores (Bass/Tile).

Problem: inputs [B=4, S=2048, D=1024] f32; WK/WQ/WV [1024, 1024] f32.
  K/Q/V = X @ W*; scores = Q K^T / sqrt(D); causal mask; softmax; out = P V.

Sharding: 8 cores = (batch b, half h); core (b, h) owns q-tile
2j + (h if j even else 1-h) in slot j (balances causal work); slot j
covers k-tiles 0..2j+1 on every core (identical SPMD program), with the
exact causal boundary enforced by a per-core, slot-parity-selected
additive mask accumulated into the scores PSUM via an identity-stationary
matmul.

Projection/exchange layout (vs the earlier V-full-recompute design):
  - Each core projects only its OWN half of K (KT columns) and V (k-row
    tiles) from a host-supplied own-half X^T (static addressing), plus Q
    for its own q-rows.  That is the minimal data-parallel share: 3 x 64k
    PE cycles instead of 4 x 64k (V was previously recomputed in full on
    both cores of a pair to dodge a second collective).
  - Halves are exchanged with FOUR 1MB intra-pair AllGathers (K in two
    pieces, V in two pieces), launched as soon as each quarter of the
    projection work is evicted.  Pair AllGathers measure ~14us/MB+5us on
    this fabric and serialize on the single gpsimd collective queue, so
    the split keeps the queue busy from ~12us in and the last piece lands
    before the attention loop needs it.
  - Attention runs slots in ASCENDING size order so scores/PV consume
    KT/V tiles in the same order the gathered loads stream them in
    (largest-first would need all 16 V tiles for the very first PV).
  - P^T for the PV matmuls uses PE transpose matmuls interleaved into
    the next slot's score matmuls (DMA XBAR transposes were tried and
    regressed ~25%: the XBAR cannot keep 72 [128,128] transposes off the
    PV critical path).

Matmuls run in bf16 with fp32 PSUM accumulation; softmax runs without
max-subtraction, row sums come free from the Exp activation's accum_out,
and normalization is folded into the output PSUM eviction as a per-
partition scale.  The output is stored bf16 and upcast on the host.

Measured (K-repeat slope, pipelined dispatch, 8 axon-tunneled NC_v3
cores): sustained slope is environment-noisy (+-15%) and statistically
tied with the previous 420k-cycle kernel (~190-210us medians for both);
per-core PE work drops 420k -> 346k cycles (~108us at the 3.2GHz burst
clock vs ~131us), so one-shot/burst executions are ~15-18% faster.
"""

from concourse.bass_utils import run_bass_kernel_spmd


from contextlib import ExitStack
from math import ceil

import ml_dtypes
import numpy as np

import concourse.mybir as mybir
import concourse.tile as tile
from concourse import bacc
from concourse.bass import ds

BF = mybir.dt.bfloat16
F32 = mybir.dt.float32
NEG = -1e9


def build_nc_pair(
    KD=8, ED=8, n_slots=8, S=2048, chunk=512, repeat=1, groups=None,
    skip_attention=False, dma_tp=False, pp_bufs=6,
):
    NQ = n_slots * 128
    DO = ED * 128
    n_ktiles = S // 128
    SH = S // 2
    scale = 1.0 / np.sqrt(np.float32(KD * 128))

    nc = bacc.Bacc(None, target_bir_lowering=False, debug=False)

    xto_d = nc.dram_tensor("xto", [KD, 128, SH], BF, kind="ExternalInput")
    xtq_d = nc.dram_tensor("xtq", [KD, 128, NQ], BF, kind="ExternalInput")
    wk_d = nc.dram_tensor("wk", [KD, 128, DO], BF, kind="ExternalInput")
    wq_d = nc.dram_tensor("wq", [KD, 128, DO], BF, kind="ExternalInput")
    wv_d = nc.dram_tensor("wv", [KD, 128, DO], BF, kind="ExternalInput")
    mask_d = nc.dram_tensor("mask", [2, 128, 256], BF, kind="ExternalInput")
    out_d = nc.dram_tensor("out", [n_slots, 128, DO], BF, kind="ExternalOutput")

    ident_d = nc.inline_tensor(np.eye(128).astype(ml_dtypes.bfloat16), "ident")
    if groups is None:
        groups = [[0, 1], [2, 3], [4, 5], [6, 7]]

    with tile.TileContext(nc) as tc, ExitStack() as ctx:
      persist = ctx.enter_context(tc.tile_pool(name="persist", bufs=1))
      pp = ctx.enter_context(tc.tile_pool(name="pp", bufs=pp_bufs, space="PSUM"))
      ptp = ctx.enter_context(tc.tile_pool(name="ptp", bufs=2, space="PSUM"))
      att = ctx.enter_context(tc.tile_pool(name="att", bufs=2))
      pts = ctx.enter_context(tc.tile_pool(name="pts", bufs=36))
      dram = ctx.enter_context(tc.tile_pool(name="dram", bufs=1, space="DRAM"))
      stage = ctx.enter_context(tc.tile_pool(name="stage", bufs=6))
      for _rep in range(repeat):
        XTO = persist.tile([128, KD, SH], BF, tag="XTO")
        XTQ = persist.tile([128, KD, NQ], BF, tag="XTQ")
        WK = persist.tile([128, KD, DO], BF, tag="WK")
        WQ = persist.tile([128, KD, DO], BF, tag="WQ")
        WV = persist.tile([128, KD, DO], BF, tag="WV")
        KT = persist.tile([128, ED, S], BF, tag="KT")
        V = persist.tile([128, n_ktiles, DO], BF, tag="V")
        QT = persist.tile([128, ED, NQ], BF, tag="QT")
        maskt = persist.tile([128, 2, 256], BF, tag="maskt")
        ident = persist.tile([128, 128], BF, tag="ident")

        k_owns = [
            dram.tile([4, 128, SH], BF, tag=f"k_own{i}", name=f"k_own{i}")
            for i in range(2)
        ]
        k_alls = [
            dram.tile([2, 4, 128, SH], BF, tag=f"k_all{i}", name=f"k_all{i}")
            for i in range(2)
        ]
        v_owns = [
            dram.tile([4, 128, DO], BF, tag=f"v_own{i}", name=f"v_own{i}")
            for i in range(2)
        ]
        v_alls = [
            dram.tile([2, 4, 128, DO], BF, tag=f"v_all{i}", name=f"v_all{i}")
            for i in range(2)
        ]

        def ag(own, all_):
            nc.gpsimd.collective_compute(
                "AllGather",
                mybir.AluOpType.bypass,
                replica_groups=groups,
                ins=[own[:]],
                outs=[all_[:]],
            )

        # -- input loads --
        nc.sync.dma_start(out=ident, in_=ident_d[:])
        for p in range(2):
            nc.sync.dma_start(out=maskt[:, p, :], in_=mask_d[p])
        # K-projection inputs first (WK + X^T-own), balanced across SP/ACT
        for kd in range(KD):
            weng = nc.sync if kd % 2 == 0 else nc.scalar
            weng.dma_start(out=WK[:, kd, :], in_=wk_d[kd])
            xeng = nc.scalar if kd % 2 == 0 else nc.sync
            xeng.dma_start(out=XTO[:, kd, :], in_=xto_d[kd])
        for kd in range(KD):
            nc.sync.dma_start(out=WV[:, kd, :], in_=wv_d[kd])
        for kd in range(KD):
            nc.sync.dma_start(out=WQ[:, kd, :], in_=wq_d[kd])
            nc.sync.dma_start(out=XTQ[:, kd, :], in_=xtq_d[kd])

        # -- K-half: KT_own[m] = WK[:, :, m-tile].T @ X_own; first AG after
        # half the e-tiles are evicted so the collective queue starts early
        for m in range(ED):
            for c in range(2):
                ps = pp.tile([128, chunk], F32, tag="pp", name="pp")
                for kd in range(KD):
                    nc.tensor.matmul(
                        ps,
                        WK[:, kd, m * 128 : (m + 1) * 128],
                        XTO[:, kd, c * chunk : (c + 1) * chunk],
                        start=(kd == 0),
                        stop=(kd == KD - 1),
                    )
                st = stage.tile([128, chunk], BF, tag="stage", name="stage")
                nc.vector.tensor_copy(st, ps)
                nc.scalar.dma_start(
                    out=k_owns[m // 4][m % 4, :, c * chunk : (c + 1) * chunk],
                    in_=st,
                )
            if m == 3:
                ag(k_owns[0], k_alls[0])
        ag(k_owns[1], k_alls[1])

        # -- V-half: V_own[t] = X_own[:, t-tile].T @ WV --
        ndc = ceil(DO / chunk)
        for t in range(SH // 128):
            psums = [
                pp.tile([128, chunk], F32, tag="pp", name="pp")
                for _ in range(ndc)
            ]
            for kd in range(KD):
                lhsT = XTO[:, kd, t * 128 : (t + 1) * 128]
                for cc in range(ndc):
                    nc.tensor.matmul(
                        psums[cc],
                        lhsT,
                        WV[:, kd, cc * chunk : (cc + 1) * chunk],
                        start=(kd == 0),
                        stop=(kd == KD - 1),
                    )
            for cc in range(ndc):
                st = stage.tile([128, chunk], BF, tag="stage", name="stage")
                nc.vector.tensor_copy(st, psums[cc])
                nc.scalar.dma_start(
                    out=v_owns[t // 4][t % 4, :, cc * chunk : (cc + 1) * chunk],
                    in_=st,
                )
            if t == 3:
                ag(v_owns[0], v_alls[0])
        ag(v_owns[1], v_alls[1])

        # -- Q projection (psum -> SBUF directly) --
        for m in range(ED):
            psums = [
                pp.tile([128, chunk], F32, tag="pp", name="pp") for _ in range(2)
            ]
            for kd in range(KD):
                lhsT = WQ[:, kd, m * 128 : (m + 1) * 128]
                for c in range(2):
                    nc.tensor.matmul(
                        psums[c],
                        lhsT,
                        XTQ[:, kd, c * chunk : (c + 1) * chunk],
                        start=(kd == 0),
                        stop=(kd == KD - 1),
                    )
            for c in range(2):
                nc.vector.tensor_copy(
                    QT[:, m, c * chunk : (c + 1) * chunk], psums[c]
                )

        # -- gathered loads (static: group position h2 == global half) --
        # KT fill: SP/ACT alternating (both idle here; exp starts later)
        for m in range(ED):
            for h2 in range(2):
                eng = nc.sync if m % 2 == 0 else nc.scalar
                eng.dma_start(
                    out=KT[:, m, h2 * SH : (h2 + 1) * SH],
                    in_=k_alls[m // 4][h2, m % 4],
                )
        # V fill in global tile order so ascending-slot attention consumes
        # while streaming; part-1 tiles (V-AG1) on SP, part-2 split SP/gpsimd
        for tg in range(n_ktiles):
            h2, t = divmod(tg, SH // 128)
            eng = nc.sync if (t < 4 or t % 2 == 1) else nc.gpsimd
            eng.dma_start(out=V[:, tg, :], in_=v_alls[t // 4][h2, t % 4])

        if skip_attention:
            for j in range(n_slots):
                ob = att.tile([128, DO], BF, tag="out")
                nc.vector.tensor_copy(ob, QT[:, j, :])
                nc.scalar.dma_start(out=out_d[j], in_=ob)
            continue

        # -- attention: 1-slot software pipeline, ascending slot order --
        def emit_transpose(P, kt, pt_tiles):
            pt_sb = pts.tile([128, 128], BF, tag="pt", name="pt")
            if dma_tp:
                nc.sync.dma_start(
                    out=pt_sb, in_=P[:, kt * 128 : (kt + 1) * 128],
                    transpose=True,
                )
            else:
                tp = ptp.tile([128, 128], BF, tag="ptp", name="ptp")
                nc.tensor.transpose(
                    tp, P[:, kt * 128 : (kt + 1) * 128], ident
                )
                nc.vector.tensor_copy(pt_sb, tp)
            pt_tiles[kt] = pt_sb

        def emit_tail(j, pt_tiles, recip):
            nkt = 2 * (j + 1)
            opsums = [
                pp.tile([128, chunk], F32, tag="pp", name="pp")
                for _ in range(ndc)
            ]
            for kt in range(nkt):
                for c in range(ndc):
                    nc.tensor.matmul(
                        opsums[c],
                        pt_tiles[kt],
                        V[:, kt, c * chunk : (c + 1) * chunk],
                        start=(kt == 0),
                        stop=(kt == nkt - 1),
                    )
            out_sb = att.tile([128, DO], BF, tag="out")
            for c in range(ndc):
                nc.scalar.mul(
                    out_sb[:, c * chunk : (c + 1) * chunk],
                    opsums[c],
                    mul=recip,
                )
            nc.scalar.dma_start(out=out_d[j], in_=out_sb)

        prev = None  # (j, P, pt_tiles, recip) of the previous slot

        slot_order = list(range(n_slots))
        for j in slot_order:
            nk = 256 * (j + 1)
            chunks = [
                (c * chunk, min(chunk, nk - c * chunk))
                for c in range(ceil(nk / chunk))
            ]
            nch = len(chunks)

            P = att.tile([128, S], BF, tag="P")
            sums = att.tile([128, 4], F32, tag="sums")

            if prev is not None:
                pj, pP, ppt, prec = prev
                pending = list(range(2 * (pj + 1)))
            else:
                pending = []

            spsums = [
                pp.tile([128, chunk], F32, tag="pp", name="pp")
                for _ in range(nch)
            ]
            for e in range(ED):
                lhsT = QT[:, e, j * 128 : (j + 1) * 128]
                for ci, (off, w) in enumerate(chunks):
                    nc.tensor.matmul(
                        spsums[ci][:, :w],
                        lhsT,
                        KT[:, e, off : off + w],
                        start=(e == 0),
                        stop=(e == ED - 1),
                    )
                    if pending:
                        emit_transpose(pP, pending.pop(0), ppt)
            loc = (nk - 256) - chunks[-1][0]
            nc.tensor.matmul(
                spsums[-1][:, loc : loc + 256],
                ident,
                maskt[:, j % 2, :],
                start=False,
                stop=False,
                skip_group_check=True,
            )
            while pending:
                emit_transpose(pP, pending.pop(0), ppt)

            for ci, (off, w) in enumerate(chunks):
                nc.scalar.activation(
                    P[:, off : off + w],
                    spsums[ci][:, :w],
                    mybir.ActivationFunctionType.Exp,
                    scale=float(scale),
                    accum_out=sums[:, ci : ci + 1],
                )

            total = att.tile([128, 1], F32, tag="total")
            nc.vector.reduce_sum(total, sums[:, :nch], axis=mybir.AxisListType.X)
            recip = att.tile([128, 1], F32, tag="recip")
            nc.vector.reciprocal(recip, total)

            if prev is not None:
                emit_tail(prev[0], prev[2], prev[3])

            pt_tiles = [None] * (2 * (j + 1))
            prev = (j, P, pt_tiles, recip)

        # drain: last slot -- interleave transposes into PV
        pj, pP, ppt, prec = prev
        nkt = 2 * (pj + 1)
        for kt in range(min(2, nkt)):
            emit_transpose(pP, kt, ppt)
        opsums = [
            pp.tile([128, chunk], F32, tag="pp", name="pp") for _ in range(ndc)
        ]
        for kt in range(nkt):
            for c in range(ndc):
                nc.tensor.matmul(
                    opsums[c],
                    ppt[kt],
                    V[:, kt, c * chunk : (c + 1) * chunk],
                    start=(kt == 0),
                    stop=(kt == nkt - 1),
                )
            if kt + 2 < nkt:
                emit_transpose(pP, kt + 2, ppt)
        out_sb = att.tile([128, DO], BF, tag="out")
        for c in range(ndc):
            nc.scalar.mul(
                out_sb[:, c * chunk : (c + 1) * chunk], opsums[c], mul=prec
            )
        nc.scalar.dma_start(out=out_d[pj], in_=out_sb)

    nc.compile()
    return nc


def host_inputs_for_core_pair(X, WKn, WQn, WVn, core, n_slots):
    b, h = core // 2, core % 2
    S = X.shape[1]
    D = X.shape[2]
    KD = D // 128
    NQ = n_slots * 128
    SH = S // 2
    bf = ml_dtypes.bfloat16

    qtiles = [2 * j + (h if j % 2 == 0 else 1 - h) for j in range(n_slots)]
    qrows = np.concatenate([np.arange(t * 128, (t + 1) * 128) for t in qtiles])

    xto = np.ascontiguousarray(
        X[b][h * SH : (h + 1) * SH].T.astype(bf)
    ).reshape(KD, 128, SH)
    xtq = np.ascontiguousarray(X[b][qrows].T.astype(bf)).reshape(KD, 128, NQ)

    def wtile(W):
        return np.ascontiguousarray(W.astype(bf)).reshape(KD, 128, -1)

    r = np.arange(128)
    tri = np.where(r[None, :] <= r[:, None], 0.0, NEG).astype(np.float32)
    mA = np.zeros((128, 256), dtype=np.float32)
    mA[:, 128:] = tri  # diagonal tile is the last covered tile
    mB = np.zeros((128, 256), dtype=np.float32)
    mB[:, :128] = tri  # diagonal tile is second-from-last; last fully masked
    mB[:, 128:] = NEG
    # slot parity p uses mask (h==0: [B, A][p], h==1: [A, B][p])
    mask = np.stack([mB, mA] if h == 0 else [mA, mB])
    return {
        "xto": xto,
        "xtq": xtq,
        "wk": wtile(WKn),
        "wq": wtile(WQn),
        "wv": wtile(WVn),
        "mask": mask.astype(bf),
    }


B, S, D_IN, D_OUT = 4, 2048, 1024, 1024
N_SLOTS = 8

_NC_CACHE = []


def _get_nc():
    if not _NC_CACHE:
        _NC_CACHE.append(build_nc_pair())
    return _NC_CACHE[0]


def _host_inputs_for_core(X, WKn, WQn, WVn, core):
    return host_inputs_for_core_pair(X, WKn, WQn, WVn, core, N_SLOTS)


def build_nc(repeat=1):
    return build_nc_pair(repeat=repeat)


def kernel(inputs, WK, WQ, WV):
    X = np.asarray(inputs, dtype=np.float32)
    WKn = np.asarray(WK, dtype=np.float32)
    WQn = np.asarray(WQ, dtype=np.float32)
    WVn = np.asarray(WV, dtype=np.float32)

    nc = _get_nc()
    in_maps = [_host_inputs_for_core(X, WKn, WQn, WVn, c) for c in range(8)]
    res = run_bass_kernel_spmd(nc, in_maps, core_ids=list(range(8)))

    out = np.zeros((B, S, D_OUT), dtype=np.float32)
    for core in range(8):
        b, h = core // 2, core % 2
        o = np.asarray(res.results[core]["out"], dtype=np.float32)
        for j in range(N_SLOTS):
            t = 2 * j + (h if j % 2 == 0 else 1 - h)
            out[b, t * 128 : (t + 1) * 128, :] = o[j]
    return out
